# revision 1
# baseline (speedup 1.0000x reference)
"""Trainium2 Bass kernel for nn_CausalSelfAttention_74268574482879.

The reference module's attention scores are overwritten by the causal mask
(q/k are discarded), so softmax weights are uniform over positions <= t:
    y = cummean_T(x) @ W_v @ W_p,   W_v = w_attn[:, 1024:1536]

Distribution: the 4096 rows of (B*T) are split into 8 chunks of 512 rows,
one per NeuronCore.  The only cross-chunk dependency is the column-sum of
all preceding rows in the same batch element; the host passes that tiny
(512,) halo vector per core while slicing the shards.

Per-core dataflow (matmuls keep operands in natural layout — the PE's
implicit transpose of the stationary operand does all layout work):
  stage A: lhsT=x_tile, rhs=U_scaled  ->  psA = scale*(local cumsum)^T (PSUM)
           tile colsums are recovered from psA's last column (one fused
           tensor_scalar each), then a DVE/GpSimd carry adds P_j[c]*scale[t]
  stage B: lhsT=Wv,     rhs=A^T       ->  M1^T = (A @ Wv)^T
  stage C: lhsT=M1^T,   rhs=Wp        ->  Y = M1 @ Wp   (natural, DMA out)
"""

import numpy as np

import concourse.bass as bass
import concourse.bacc as bacc
import concourse.mybir as mybir
import concourse.tile as tile
from concourse import bass_utils

N_CORES = 8
B, T, C = 2, 2048, 512
CHUNK = 512               # rows of flattened (B*T) per core
P = 128
NT = CHUNK // P           # 4 row-tiles per chunk
NI = C // P               # 4 col-tiles of the 512 feature dim
F32 = mybir.dt.float32
F32R = mybir.dt.float32r
BF16 = mybir.dt.bfloat16

MODE = ["f32r"]           # "f32" | "f32r" | "bf16" (stage B/C dtype)
TRACE = [False]
LAST_RESULT = [None]
_STATE = {}


def _build_nc(mode):
    nc = bacc.Bacc(
        "TRN2", target_bir_lowering=False, debug=False, num_devices=N_CORES
    )
    # dtype plan per mode: a_dt feeds stage-A matmuls, bc_dt feeds B/C.
    # float32r keeps fp32 bits but runs the PE in single-pass reduced mode;
    # the verifier wants every producer of a matmul operand to declare it.
    if mode == "f32":
        a_dt, bc_dt = F32, F32
    elif mode == "f32r":
        a_dt, bc_dt = F32R, F32R
    else:  # bf16 B/C, f32r stage A
        a_dt, bc_dt = F32R, BF16
    bc_bf16 = bc_dt == BF16
    wdma_dt = F32 if bc_bf16 else bc_dt

    x_d = nc.dram_tensor("x", (CHUNK, C), a_dt, kind="ExternalInput")
    wv_d = nc.dram_tensor("wv", (C, C), wdma_dt, kind="ExternalInput")
    wp_d = nc.dram_tensor("wp", (C, C), wdma_dt, kind="ExternalInput")
    us_d = nc.dram_tensor("us", (P, P), a_dt, kind="ExternalInput")
    sc_d = nc.dram_tensor("sc", (P, NI + NT), F32, kind="ExternalInput")
    y_d = nc.dram_tensor("y", (CHUNK, C), F32, kind="ExternalOutput")

    x_ap, wv_ap, wp_ap = x_d.ap(), wv_d.ap(), wp_d.ap()
    us_ap, sc_ap, y_ap = us_d.ap(), sc_d.ap(), y_d.ap()

    with tile.TileContext(nc) as tc:
        with (
            tc.tile_pool(name="io", bufs=1) as io,
            tc.tile_pool(name="tmp", bufs=4) as tmp_pool,
            tc.tile_pool(name="psbig", bufs=2, space="PSUM") as ps_pool,
        ):
            # ---- inputs to SBUF (order = DMA priority) ----
            # x arrives as column-slices aligned to the i-rounds: slice i
            # holds all 512 rows of features ci as (P, NT, P)
            x_r = x_ap.rearrange("(j p) c -> p j c", p=P)
            us_sb = io.tile([P, P], a_dt, name="us_sb")
            nc.gpsimd.dma_start(us_sb[:], us_ap[:, :])
            xc = []
            for i in range(NI):
                t = io.tile([P, NT, P], a_dt, name=f"xc{i}")
                eng = nc.sync if i % 2 == 0 else nc.gpsimd
                eng.dma_start(t[:], x_r[:, :, i * P : (i + 1) * P])
                xc.append(t)
                if i == 0:
                    # pc | scv: prefix colsums and the final 1/(t+1) column
                    cs_sb = io.tile([P, NI + NT], F32, name="cs_sb")
                    nc.sync.dma_start(cs_sb[:], sc_ap[:, :])
            pc_sb = cs_sb[:, 0:NI]
            scv_sb = cs_sb[:, NI : NI + NT]
            wv_pack = io.tile([P, NI, C], wdma_dt, name="wv_pack")
            nc.sync.dma_start(wv_pack[:], wv_ap.rearrange("(k p) c -> p k c", p=P))
            wp_pack = io.tile([P, NI, C], wdma_dt, name="wp_pack")
            nc.sync.dma_start(wp_pack[:], wp_ap.rearrange("(k p) c -> p k c", p=P))
            wv_sb = [wv_pack[:, i, :] for i in range(NI)]
            wp_sb = [wp_pack[:, j, :] for j in range(NI)]

            if bc_bf16:
                wvb, wpb = [], []
                for i in range(NI):
                    t = io.tile([P, C], BF16, name=f"wvb{i}")
                    nc.scalar.copy(t[:], wv_sb[i][:])
                    wvb.append(t)
                for j in range(NI):
                    t = io.tile([P, C], BF16, name=f"wpb{j}")
                    nc.scalar.copy(t[:], wp_sb[j][:])
                    wpb.append(t)
            else:
                wvb, wpb = wv_sb, wp_sb

            # ---- stage A: raw local cumsum; i-outer rounds so A_sb[i]
            # completes early and stage B overlaps.  The 1/(t+1) scale is
            # deferred all the way to the Y eviction (it commutes) ----
            Pc_sb = io.tile([P, NT * NI], F32, name="Pc_sb")
            A_sb = [
                io.tile([P, CHUNK], bc_dt, name=f"A{i}") for i in range(NI)
            ]
            for i in range(NI):
                nc.vector.tensor_copy(
                    Pc_sb[:, i * NT : i * NT + 1], pc_sb[:, i : i + 1]
                )
                psA = []
                for j in range(NT):
                    pa = ps_pool.tile(
                        [P, P], F32, name=f"psA{i}_{j}", tag="small", bufs=6
                    )
                    nc.tensor.matmul(
                        pa[:], xc[i][:, j, :], us_sb[:], start=True, stop=True
                    )
                    psA.append(pa)
                for j in range(NT):
                    col = i * NT + j
                    if j + 1 < NT:
                        # running prefix: next = cur + colsum_j (psA last col)
                        nc.vector.tensor_add(
                            Pc_sb[:, col + 1 : col + 2],
                            Pc_sb[:, col : col + 1],
                            psA[j][:, P - 1 : P],
                        )
                    nc.vector.tensor_scalar_add(
                        A_sb[i][:, j * P : (j + 1) * P],
                        psA[j][:],
                        Pc_sb[:, col : col + 1],
                    )

            # ---- stage B: M1^T = (A @ Wv)^T ----
            M1_sb = []
            for jj in range(NI):
                psm = ps_pool.tile([P, CHUNK], F32, name=f"psM{jj}", tag="big")
                cj = slice(jj * P, (jj + 1) * P)
                for i in range(NI):
                    nc.tensor.matmul(
                        psm[:],
                        wvb[i][:, cj],
                        A_sb[i][:],
                        start=(i == 0),
                        stop=(i == NI - 1),
                    )
                m1 = io.tile([P, CHUNK], bc_dt, name=f"M1{jj}")
                nc.vector.tensor_copy(m1[:], psm[:])
                M1_sb.append(m1)

            # ---- stage C: Y = M1 @ Wp  (natural layout) ----
            for tt in range(NT):
                psy = ps_pool.tile([P, C], F32, name=f"psY{tt}", tag="big")
                st = slice(tt * P, (tt + 1) * P)
                for jj in range(NI):
                    nc.tensor.matmul(
                        psy[:],
                        M1_sb[jj][:, st],
                        wpb[jj][:],
                        start=(jj == 0),
                        stop=(jj == NI - 1),
                    )
                ysb = io.tile([P, C], F32, name=f"y{tt}")
                nc.vector.tensor_scalar_mul(
                    ysb[:], psy[:], scv_sb[:, tt : tt + 1]
                )
                nc.sync.dma_start(y_ap[st, :], ysb[:])

    nc.compile()
    return nc


def _get_nc():
    key = MODE[0]
    if key not in _STATE:
        _STATE[key] = _build_nc(key)
    return _STATE[key]


def _prepare_in_maps(x, w_attn, w_proj):
    x = np.asarray(x, dtype=np.float32)
    w_attn = np.asarray(w_attn, dtype=np.float32)
    w_proj = np.ascontiguousarray(np.asarray(w_proj, dtype=np.float32))
    wv = np.ascontiguousarray(w_attn[:, 2 * C : 3 * C])

    in_maps = []
    for core in range(N_CORES):
        b, tc = divmod(core, T // CHUNK)
        goff = tc * CHUNK
        chunk = np.ascontiguousarray(x[b, goff : goff + CHUNK, :])
        # halo: column-sum of all earlier rows in this batch element
        p = x[b, :goff, :].sum(axis=0, dtype=np.float32) if goff else np.zeros(
            C, np.float32
        )
        # scv[r, tt] = 1/(global_row+1) for row tt*P + r of this chunk
        scale = (1.0 / (goff + np.arange(1, CHUNK + 1))).astype(np.float32)
        sc = np.concatenate(
            [p.reshape(NI, P).T, scale.reshape(NT, P).T], axis=1
        ).astype(np.float32)
        us = np.triu(np.ones((P, P), np.float32))  # s <= t
        in_maps.append(
            {"x": chunk, "wv": wv, "wp": w_proj, "us": us, "sc": sc}
        )
    return in_maps


def kernel(x, w_attn, w_proj):
    nc = _get_nc()
    in_maps = _prepare_in_maps(x, w_attn, w_proj)
    res = bass_utils.run_bass_kernel_spmd(
        nc, in_maps, core_ids=list(range(N_CORES)), trace=TRACE[0]
    )
    LAST_RESULT[0] = res
    y = np.empty((B, T, C), np.float32)
    for core in range(N_CORES):
        b, tc = divmod(core, T // CHUNK)
        y[b, tc * CHUNK : (tc + 1) * CHUNK, :] = res.results[core]["y"]
    return y



# revision 2
# speedup vs baseline: 1.4584x; 1.4584x over previous
"""Trainium2 Bass kernel for nn_CausalSelfAttention_74268574482879.

The reference module's attention scores are overwritten by the causal mask
(q/k are discarded), so softmax weights are uniform over positions <= t:
    y = cummean_T(x) @ W,   W = w_attn[:, 1024:1536] @ w_proj  (host-folded)

Distribution: the 4096 rows of (B*T) are split into 8 chunks of 512 rows,
one per NeuronCore.  The only cross-chunk dependency is the column-sum of
all preceding rows in the same batch element; the host passes that tiny
(512,) halo vector per core while slicing the shards.

Per-core dataflow (~34 instructions):
  - x^T arrives pre-transposed/packed (features on partitions, bf16)
  - 4 DVE tensor_tensor_scan ops compute the running column-sum along the
    free (time) axis, seeded with the halo via `initial` -> A^T in SBUF
  - 16 bf16 matmuls: psY_j += A^T[ci, tj]^T @ W[ci, :]  (i-outer so each
    scan's output streams into the PE while later x tiles are still in DMA)
  - eviction fuses the deferred 1/(t+1) row scale (per-partition scalar),
    alternating DVE / Activation engines, then DMA out per row-tile
"""

import numpy as np
from ml_dtypes import bfloat16

import concourse.bass as bass
import concourse.bacc as bacc
import concourse.mybir as mybir
import concourse.tile as tile
from concourse import bass_utils

N_CORES = 8
B, T, C = 2, 2048, 512
CHUNK = 512               # rows of flattened (B*T) per core
P = 128
NT = CHUNK // P           # 4 row-tiles per chunk
NI = C // P               # 4 col-tiles of the 512 feature dim
F32 = mybir.dt.float32
BF16 = mybir.dt.bfloat16

MODE = ["bf16"]           # "bf16" (f32 out) | "bf16o" (bf16 out)
TRACE = [False]
LAST_RESULT = [None]
_STATE = {}


def _build_nc(mode):
    out_bf16 = mode == "bf16o"
    y_dt = BF16 if out_bf16 else F32

    nc = bacc.Bacc(
        "TRN2", target_bir_lowering=False, debug=False, num_devices=N_CORES
    )

    xt_d = nc.dram_tensor("xt", (P, NI, CHUNK), BF16, kind="ExternalInput")
    w_d = nc.dram_tensor("w", (P, NI, C), BF16, kind="ExternalInput")
    sc_d = nc.dram_tensor("sc", (P, NI + NT), F32, kind="ExternalInput")
    y_d = nc.dram_tensor("y", (CHUNK, C), y_dt, kind="ExternalOutput")

    xt_ap, w_ap, sc_ap, y_ap = xt_d.ap(), w_d.ap(), sc_d.ap(), y_d.ap()
    ADD = mybir.AluOpType.add
    BYP = mybir.AluOpType.bypass

    with tile.TileContext(nc) as tc:
        with (
            tc.tile_pool(name="io", bufs=1) as io,
            tc.tile_pool(name="ps", bufs=1, space="PSUM") as psp,
        ):
            # ---- inputs to SBUF; xt tiles land one at a time so the scan
            # and PE pipeline starts before the full chunk has arrived ----
            sc_sb = io.tile([P, NI + NT], F32, name="sc_sb")
            nc.sync.dma_start(sc_sb[:], sc_ap[:, :])
            xt_sb = io.tile([P, NI, CHUNK], BF16, name="xt_sb")
            for i in range(NI):
                nc.sync.dma_start(xt_sb[:, i, :], xt_ap[:, i, :])
            w_sb = io.tile([P, NI, C], BF16, name="w_sb")
            nc.gpsimd.dma_start(w_sb[:, 0:2, :], w_ap[:, 0:2, :])
            nc.gpsimd.dma_start(w_sb[:, 2:4, :], w_ap[:, 2:4, :])

            # ---- stage A: A^T[ci, t] = halo_ci + cumsum_t x^T[ci, t] ----
            A_sb = io.tile([P, NI, CHUNK], BF16, name="A_sb")
            for i in range(NI):
                nc.vector.tensor_tensor_scan(
                    A_sb[:, i, :],
                    xt_sb[:, i, :],
                    xt_sb[:, i, :],
                    sc_sb[:, i : i + 1],
                    ADD,
                    BYP,
                )

            # ---- stage Y: psY_j = sum_i A^T[ci, tj]^T @ W[ci, :] ----
            psy = [
                psp.tile([P, C], F32, name=f"psy{j}", tag=f"psy{j}")
                for j in range(NT)
            ]
            for i in range(NI):
                for j in range(NT):
                    nc.tensor.matmul(
                        psy[j][:],
                        A_sb[:, i, j * P : (j + 1) * P],
                        w_sb[:, i, :],
                        start=(i == 0),
                        stop=(i == NI - 1),
                    )

            # ---- eviction: fuse the 1/(t+1) row scale; DVE and ACT
            # alternate so neither serializes the tail ----
            for j in range(NT):
                ysb = io.tile([P, C], y_dt, name=f"y{j}")
                scol = sc_sb[:, NI + j : NI + j + 1]
                if j % 2 == 0:
                    nc.vector.tensor_scalar_mul(ysb[:], psy[j][:], scol)
                else:
                    nc.scalar.mul(ysb[:], psy[j][:], scol)
                eng = nc.sync if j % 2 == 0 else nc.gpsimd
                eng.dma_start(y_ap[j * P : (j + 1) * P, :], ysb[:])

    nc.compile()
    return nc


def _get_nc():
    key = MODE[0]
    if key not in _STATE:
        _STATE[key] = _build_nc(key)
    return _STATE[key]


def _prepare_in_maps(x, w_attn, w_proj):
    x = np.asarray(x, dtype=np.float32)
    w_attn = np.asarray(w_attn, dtype=np.float32)
    w_proj = np.asarray(w_proj, dtype=np.float32)
    w = (w_attn[:, 2 * C : 3 * C] @ w_proj).astype(np.float32)
    wpk = np.ascontiguousarray(
        w.reshape(NI, P, C).transpose(1, 0, 2)
    ).astype(bfloat16)

    in_maps = []
    for core in range(N_CORES):
        b, tc = divmod(core, T // CHUNK)
        goff = tc * CHUNK
        chunk = x[b, goff : goff + CHUNK, :]
        # (P, NI, CHUNK): features on partitions, time on the free axis
        xt = np.ascontiguousarray(
            chunk.T.reshape(NI, P, CHUNK).transpose(1, 0, 2)
        ).astype(bfloat16)
        # halo: column-sum of all earlier rows in this batch element
        p = x[b, :goff, :].sum(axis=0, dtype=np.float32) if goff else np.zeros(
            C, np.float32
        )
        # scv[r, tt] = 1/(global_row+1) for row tt*P + r of this chunk
        scale = (1.0 / (goff + np.arange(1, CHUNK + 1))).astype(np.float32)
        sc = np.concatenate(
            [p.reshape(NI, P).T, scale.reshape(NT, P).T], axis=1
        ).astype(np.float32)
        in_maps.append({"xt": xt, "w": wpk, "sc": sc})
    return in_maps


def kernel(x, w_attn, w_proj):
    nc = _get_nc()
    in_maps = _prepare_in_maps(x, w_attn, w_proj)
    res = bass_utils.run_bass_kernel_spmd(
        nc, in_maps, core_ids=list(range(N_CORES)), trace=TRACE[0]
    )
    LAST_RESULT[0] = res
    y = np.empty((B, T, C), np.float32)
    for core in range(N_CORES):
        b, tc = divmod(core, T // CHUNK)
        y[b, tc * CHUNK : (tc + 1) * CHUNK, :] = np.asarray(
            res.results[core]["y"], dtype=np.float32
        )
    return y


# revision 6
# speedup vs baseline: 1.5066x; 1.0331x over previous
"""Trainium2 Bass kernel for nn_CausalSelfAttention_74268574482879.

The reference module's attention scores are overwritten by the causal mask
(q/k are discarded), so softmax weights are uniform over positions <= t:
    y = cummean_T(x) @ W,   W = w_attn[:, 1024:1536] @ w_proj  (host-folded)

Distribution: the 4096 rows of (B*T) are split into 8 chunks of 512 rows,
one per NeuronCore.  The only cross-chunk dependency is the column-sum of
all preceding rows in the same batch element; the host passes that tiny
(512,) halo vector per core while slicing the shards.

Per-core dataflow (~40 instructions), tuned for DMA issue rate (the
per-queue descriptor rate, not HBM bandwidth, limits transfers) and for
dependency depth:
  - x^T arrives pre-transposed/packed bf16 (features on partitions), one
    DMA per 128-feature block spread over three trigger queues; W is a
    single 4 KB/partition DMA on the scalar queue
  - 8 half-length tensor_tensor_scan ops (DVE gets blocks 0-1, GpSimd
    blocks 2-3) compute the running column-sum along time, seeded with
    the halo via `initial`, halves chained via initial=prev[:, -1:]
  - 16 bf16 matmuls accumulate psY_j; emission order follows data
    availability (first-half time tiles first, scan block 3 last) so the
    PE never waits long and output row-pairs finish early
  - eviction fuses the deferred 1/(t+1) row scale (per-partition scalar),
    split across Activation/DVE; y leaves as two packed bf16 DMAs that
    the host unpacks/upcasts
"""

import numpy as np
from ml_dtypes import bfloat16

import concourse.bass as bass
import concourse.bacc as bacc
import concourse.mybir as mybir
import concourse.tile as tile
from concourse import bass_utils

N_CORES = 8
B, T, C = 2, 2048, 512
CHUNK = 512               # rows of flattened (B*T) per core
P = 128
NT = CHUNK // P           # 4 row-tiles per chunk
NI = C // P               # 4 col-tiles of the 512 feature dim
H = CHUNK // 2            # half-scan length
F32 = mybir.dt.float32
BF16 = mybir.dt.bfloat16

MODE = ["bf16o"]          # "bf16o" (bf16 out) | "bf16" (f32 out)
TRACE = [False]
LAST_RESULT = [None]
_STATE = {}


def _build_nc(mode):
    out_bf16 = mode == "bf16o"
    y_dt = BF16 if out_bf16 else F32

    nc = bacc.Bacc(
        "TRN2", target_bir_lowering=False, debug=False, num_devices=N_CORES
    )

    xt_d = nc.dram_tensor("xt", (P, NI, CHUNK), BF16, kind="ExternalInput")
    w_d = nc.dram_tensor("w", (P, NI, C), BF16, kind="ExternalInput")
    sc_d = nc.dram_tensor("sc", (P, NI + NT), F32, kind="ExternalInput")
    # y is packed partition-major ([p, j, c] = row j*P+p); host unpacks
    y_d = nc.dram_tensor("y", (P, NT, C), y_dt, kind="ExternalOutput")

    xt_ap, w_ap, sc_ap, y_ap = xt_d.ap(), w_d.ap(), sc_d.ap(), y_d.ap()
    ADD = mybir.AluOpType.add
    BYP = mybir.AluOpType.bypass

    with tile.TileContext(nc) as tc:
        with (
            tc.tile_pool(name="io", bufs=1) as io,
            tc.tile_pool(name="ps", bufs=1, space="PSUM") as psp,
        ):
            # ---- inputs; xt blocks spread over three queues so scans
            # start as early as the per-queue packet rate allows ----
            sc_sb = io.tile([P, NI + NT], F32, name="sc_sb")
            xt_sb = io.tile([P, NI, CHUNK], BF16, name="xt_sb")
            w_sb = io.tile([P, NI, C], BF16, name="w_sb")
            nc.sync.dma_start(sc_sb[:], sc_ap[:, :])
            nc.sync.dma_start(xt_sb[:, 0, :], xt_ap[:, 0, :])
            nc.sync.dma_start(xt_sb[:, 1, :], xt_ap[:, 1, :])
            nc.gpsimd.dma_start(xt_sb[:, 2, :], xt_ap[:, 2, :])
            nc.scalar.dma_start(w_sb[:], w_ap[:, :, :])
            nc.scalar.dma_start(xt_sb[:, 3, :], xt_ap[:, 3, :])

            # ---- stage A: A^T[ci, t] = halo_ci + cumsum_t x^T[ci, t],
            # as chained half-scans on both vector engines ----
            A_sb = io.tile([P, NI, CHUNK], BF16, name="A_sb")
            for i in (0, 2, 1, 3):   # xt arrival order across the queues
                eng = nc.vector
                eng.tensor_tensor_scan(
                    A_sb[:, i, 0:H],
                    xt_sb[:, i, 0:H],
                    xt_sb[:, i, 0:H],
                    sc_sb[:, i : i + 1],
                    ADD,
                    BYP,
                )
                eng.tensor_tensor_scan(
                    A_sb[:, i, H:CHUNK],
                    xt_sb[:, i, H:CHUNK],
                    xt_sb[:, i, H:CHUNK],
                    A_sb[:, i, H - 1 : H],
                    ADD,
                    BYP,
                )

            # ---- stage Y: psY_j = sum_i A^T[ci, tj]^T @ W[ci, :] ----
            psy = [
                psp.tile([P, C], F32, name=f"psy{j}", tag=f"psy{j}")
                for j in range(NT)
            ]
            I_ORDER = (0, 2, 1)   # i=3 rounds are emitted last per j-pair

            def mm(j, i):
                nc.tensor.matmul(
                    psy[j][:],
                    A_sb[:, i, j * P : (j + 1) * P],
                    w_sb[:, i, :],
                    start=(i == 0),
                    stop=(i == NI - 1),
                )

            y_sb = io.tile([P, NT, C], y_dt, name="y_sb")

            def evict(j):
                scol = sc_sb[:, NI + j : NI + j + 1]
                if j < 2:
                    nc.scalar.mul(y_sb[:, j, :], psy[j][:], scol)
                else:
                    nc.vector.tensor_scalar_mul(y_sb[:, j, :], psy[j][:], scol)

            for i in I_ORDER:
                mm(0, i)
                mm(1, i)
                mm(2, i)
                mm(3, i)
            for j in (0, 1):
                mm(j, 3)
                evict(j)
            nc.sync.dma_start(y_ap[:, 0:2, :], y_sb[:, 0:2, :])
            for j in (2, 3):
                mm(j, 3)
                evict(j)
            nc.gpsimd.dma_start(y_ap[:, 2:4, :], y_sb[:, 2:4, :])

    nc.compile()
    return nc


def _get_nc():
    key = MODE[0]
    if key not in _STATE:
        _STATE[key] = _build_nc(key)
    return _STATE[key]


def _prepare_in_maps(x, w_attn, w_proj):
    x = np.asarray(x, dtype=np.float32)
    w_attn = np.asarray(w_attn, dtype=np.float32)
    w_proj = np.asarray(w_proj, dtype=np.float32)
    w = (w_attn[:, 2 * C : 3 * C] @ w_proj).astype(np.float32)
    wpk = np.ascontiguousarray(
        w.reshape(NI, P, C).transpose(1, 0, 2)
    ).astype(bfloat16)

    in_maps = []
    for core in range(N_CORES):
        b, tc = divmod(core, T // CHUNK)
        goff = tc * CHUNK
        chunk = x[b, goff : goff + CHUNK, :]
        # (P, NI, CHUNK): features on partitions, time on the free axis
        xt = np.ascontiguousarray(
            chunk.T.reshape(NI, P, CHUNK).transpose(1, 0, 2)
        ).astype(bfloat16)
        # halo: column-sum of all earlier rows in this batch element
        p = x[b, :goff, :].sum(axis=0, dtype=np.float32) if goff else np.zeros(
            C, np.float32
        )
        # scv[r, tt] = 1/(global_row+1) for row tt*P + r of this chunk
        scale = (1.0 / (goff + np.arange(1, CHUNK + 1))).astype(np.float32)
        sc = np.concatenate(
            [p.reshape(NI, P).T, scale.reshape(NT, P).T], axis=1
        ).astype(np.float32)
        in_maps.append({"xt": xt, "w": wpk, "sc": sc})
    return in_maps


def kernel(x, w_attn, w_proj):
    nc = _get_nc()
    in_maps = _prepare_in_maps(x, w_attn, w_proj)
    res = bass_utils.run_bass_kernel_spmd(
        nc, in_maps, core_ids=list(range(N_CORES)), trace=TRACE[0]
    )
    LAST_RESULT[0] = res
    y = np.empty((B, T, C), np.float32)
    for core in range(N_CORES):
        b, tc = divmod(core, T // CHUNK)
        yp = np.asarray(res.results[core]["y"], dtype=np.float32)
        y[b, tc * CHUNK : (tc + 1) * CHUNK, :] = yp.transpose(1, 0, 2).reshape(
            CHUNK, C
        )
    return y


# revision 8
# speedup vs baseline: 1.5476x; 1.0272x over previous
"""Trainium2 Bass kernel for nn_CausalSelfAttention_74268574482879.

The reference module's attention scores are overwritten by the causal mask
(q/k are discarded), so softmax weights are uniform over positions <= t:
    y = cummean_T(x) @ W,   W = w_attn[:, 1024:1536] @ w_proj  (host-folded)

Distribution: the 4096 rows of (B*T) are split into 8 chunks of 512 rows,
one per NeuronCore.  The only cross-chunk dependency is the column-sum of
all preceding rows in the same batch element; the host passes that tiny
(512,) halo vector per core while slicing the shards.

Per-core dataflow (~40 instructions), tuned for DMA issue rate (the
per-queue descriptor rate, not HBM bandwidth, limits transfers) and for
dependency depth:
  - x^T arrives pre-transposed/packed bf16 (features on partitions), one
    DMA per 128-feature block spread over three trigger queues; W is a
    single 4 KB/partition DMA on the scalar queue
  - 8 half-length tensor_tensor_scan ops (DVE gets blocks 0-1, GpSimd
    blocks 2-3) compute the running column-sum along time, seeded with
    the halo via `initial`, halves chained via initial=prev[:, -1:]
  - 16 bf16 matmuls accumulate psY_j; emission order follows data
    availability (first-half time tiles first, scan block 3 last) so the
    PE never waits long and output row-pairs finish early
  - eviction fuses the deferred 1/(t+1) row scale (per-partition scalar),
    split across Activation/DVE; y leaves as two packed bf16 DMAs that
    the host unpacks/upcasts
"""

import numpy as np
from ml_dtypes import bfloat16

import concourse.bass as bass
import concourse.bacc as bacc
import concourse.mybir as mybir
import concourse.tile as tile
from concourse import bass_utils

N_CORES = 8
B, T, C = 2, 2048, 512
CHUNK = 512               # rows of flattened (B*T) per core
P = 128
NT = CHUNK // P           # 4 row-tiles per chunk
NI = C // P               # 4 col-tiles of the 512 feature dim
H = CHUNK // 2            # half-scan length
F32 = mybir.dt.float32
BF16 = mybir.dt.bfloat16

MODE = ["bf16o"]          # "bf16o" (bf16 out) | "bf16" (f32 out)
TRACE = [False]
LAST_RESULT = [None]
_STATE = {}


def _build_nc(mode):
    out_bf16 = mode == "bf16o"
    y_dt = BF16 if out_bf16 else F32

    nc = bacc.Bacc(
        "TRN2", target_bir_lowering=False, debug=False, num_devices=N_CORES
    )

    xt_d = nc.dram_tensor("xt", (P, NI, CHUNK), BF16, kind="ExternalInput")
    w_d = nc.dram_tensor("w", (P, NI, C), BF16, kind="ExternalInput")
    sc_d = nc.dram_tensor("sc", (P, NI + NT), F32, kind="ExternalInput")
    # y is packed partition-major ([p, j, c] = row j*P+p); host unpacks
    y_d = nc.dram_tensor("y", (P, NT, C), y_dt, kind="ExternalOutput")

    xt_ap, w_ap, sc_ap, y_ap = xt_d.ap(), w_d.ap(), sc_d.ap(), y_d.ap()
    ADD = mybir.AluOpType.add
    BYP = mybir.AluOpType.bypass

    with tile.TileContext(nc) as tc:
        with (
            tc.tile_pool(name="io", bufs=1) as io,
            tc.tile_pool(name="ps", bufs=1, space="PSUM") as psp,
        ):
            # ---- inputs; xt blocks spread over three queues so scans
            # start as early as the per-queue packet rate allows ----
            # need-ordered streaming: each queue delivers tensors in the
            # order the pipeline consumes them (xt_i before W_i; big W
            # split per block so round 0 is not gated on all of W)
            sc_sb = io.tile([P, NI + NT], F32, name="sc_sb")
            xt_sb = io.tile([P, NI, CHUNK], BF16, name="xt_sb")
            w_sb = io.tile([P, NI, C], BF16, name="w_sb")
            nc.scalar.dma_start(sc_sb[:], sc_ap[:, :])
            nc.sync.dma_start(xt_sb[:, 0, :], xt_ap[:, 0, :])
            nc.gpsimd.dma_start(xt_sb[:, 2, :], xt_ap[:, 2, :])
            nc.scalar.dma_start(w_sb[:, 0, :], w_ap[:, 0, :])
            nc.sync.dma_start(w_sb[:, 1, :], w_ap[:, 1, :])
            nc.scalar.dma_start(xt_sb[:, 1, :], xt_ap[:, 1, :])
            nc.gpsimd.dma_start(w_sb[:, 3, :], w_ap[:, 3, :])
            nc.sync.dma_start(xt_sb[:, 3, :], xt_ap[:, 3, :])
            nc.scalar.dma_start(w_sb[:, 2, :], w_ap[:, 2, :])

            # ---- stage A: A^T[ci, t] = halo_ci + cumsum_t x^T[ci, t] ----
            A_sb = io.tile([P, NI, CHUNK], BF16, name="A_sb")
            for i in (0, 2, 1, 3):   # xt arrival order across the queues
                nc.vector.tensor_tensor_scan(
                    A_sb[:, i, :],
                    xt_sb[:, i, :],
                    xt_sb[:, i, :],
                    sc_sb[:, i : i + 1],
                    ADD,
                    BYP,
                )

            # ---- stage Y: psY_j = sum_i A^T[ci, tj]^T @ W[ci, :] ----
            psy = [
                psp.tile([P, C], F32, name=f"psy{j}", tag=f"psy{j}")
                for j in range(NT)
            ]
            I_ORDER = (0, 2, 1)   # i=3 rounds are emitted last per j-pair

            def mm(j, i):
                nc.tensor.matmul(
                    psy[j][:],
                    A_sb[:, i, j * P : (j + 1) * P],
                    w_sb[:, i, :],
                    start=(i == 0),
                    stop=(i == NI - 1),
                )

            y_sb = io.tile([P, NT, C], y_dt, name="y_sb")

            def evict(j):
                scol = sc_sb[:, NI + j : NI + j + 1]
                if j % 2 == 0:
                    nc.scalar.mul(y_sb[:, j, :], psy[j][:], scol)
                else:
                    nc.vector.tensor_scalar_mul(y_sb[:, j, :], psy[j][:], scol)

            for i in I_ORDER:
                mm(0, i)
                mm(1, i)
                mm(2, i)
                mm(3, i)
            Y_ENG = (nc.sync, nc.gpsimd, nc.scalar, nc.sync)
            for j in range(NT):
                mm(j, 3)
                evict(j)
                Y_ENG[j].dma_start(y_ap[:, j, :], y_sb[:, j, :])

    nc.compile()
    return nc


def _get_nc():
    key = MODE[0]
    if key not in _STATE:
        _STATE[key] = _build_nc(key)
    return _STATE[key]


def _prepare_in_maps(x, w_attn, w_proj):
    x = np.asarray(x, dtype=np.float32)
    w_attn = np.asarray(w_attn, dtype=np.float32)
    w_proj = np.asarray(w_proj, dtype=np.float32)
    w = (w_attn[:, 2 * C : 3 * C] @ w_proj).astype(np.float32)
    wpk = np.ascontiguousarray(
        w.reshape(NI, P, C).transpose(1, 0, 2)
    ).astype(bfloat16)

    in_maps = []
    for core in range(N_CORES):
        b, tc = divmod(core, T // CHUNK)
        goff = tc * CHUNK
        chunk = x[b, goff : goff + CHUNK, :]
        # (P, NI, CHUNK): features on partitions, time on the free axis
        xt = np.ascontiguousarray(
            chunk.T.reshape(NI, P, CHUNK).transpose(1, 0, 2)
        ).astype(bfloat16)
        # halo: column-sum of all earlier rows in this batch element
        p = x[b, :goff, :].sum(axis=0, dtype=np.float32) if goff else np.zeros(
            C, np.float32
        )
        # scv[r, tt] = 1/(global_row+1) for row tt*P + r of this chunk
        scale = (1.0 / (goff + np.arange(1, CHUNK + 1))).astype(np.float32)
        sc = np.concatenate(
            [p.reshape(NI, P).T, scale.reshape(NT, P).T], axis=1
        ).astype(np.float32)
        in_maps.append({"xt": xt, "w": wpk, "sc": sc})
    return in_maps


def kernel(x, w_attn, w_proj):
    nc = _get_nc()
    in_maps = _prepare_in_maps(x, w_attn, w_proj)
    res = bass_utils.run_bass_kernel_spmd(
        nc, in_maps, core_ids=list(range(N_CORES)), trace=TRACE[0]
    )
    LAST_RESULT[0] = res
    y = np.empty((B, T, C), np.float32)
    for core in range(N_CORES):
        b, tc = divmod(core, T // CHUNK)
        yp = np.asarray(res.results[core]["y"], dtype=np.float32)
        y[b, tc * CHUNK : (tc + 1) * CHUNK, :] = yp.transpose(1, 0, 2).reshape(
            CHUNK, C
        )
    return y


# revision 11
# speedup vs baseline: 1.6093x; 1.0399x over previous
"""Trainium2 Bass kernel for nn_CausalSelfAttention_74268574482879.

The reference module's attention scores are overwritten by the causal mask
(q/k are discarded), so softmax weights are uniform over positions <= t:
    y = cummean_T(x) @ W,   W = w_attn[:, 1024:1536] @ w_proj  (host-folded)

Distribution: the 4096 rows of (B*T) are split into 8 chunks of 512 rows,
one per NeuronCore.  The only cross-chunk dependency is the column-sum of
all preceding rows in the same batch element; the host passes that tiny
(512,) halo vector per core while slicing the shards.

Per-core dataflow (~40 instructions), tuned for DMA issue rate (the
per-queue descriptor rate, not HBM bandwidth, limits transfers) and for
dependency depth:
  - x^T arrives pre-transposed/packed bf16 (features on partitions), one
    DMA per 128-feature block spread over three trigger queues; W is a
    single 4 KB/partition DMA on the scalar queue
  - 8 half-length tensor_tensor_scan ops (DVE gets blocks 0-1, GpSimd
    blocks 2-3) compute the running column-sum along time, seeded with
    the halo via `initial`, halves chained via initial=prev[:, -1:]
  - 16 bf16 matmuls accumulate psY_j; emission order follows data
    availability (first-half time tiles first, scan block 3 last) so the
    PE never waits long and output row-pairs finish early
  - eviction fuses the deferred 1/(t+1) row scale (per-partition scalar),
    split across Activation/DVE; y leaves as two packed bf16 DMAs that
    the host unpacks/upcasts
"""

import numpy as np
from ml_dtypes import bfloat16

import concourse.bass as bass
import concourse.bacc as bacc
import concourse.mybir as mybir
import concourse.tile as tile
from concourse import bass_utils

N_CORES = 8
B, T, C = 2, 2048, 512
CHUNK = 512               # rows of flattened (B*T) per core
P = 128
NT = CHUNK // P           # 4 row-tiles per chunk
NI = C // P               # 4 col-tiles of the 512 feature dim
H = CHUNK // 2            # half-scan length
F32 = mybir.dt.float32
BF16 = mybir.dt.bfloat16

MODE = ["bf16o"]          # "bf16o" (bf16 out) | "bf16" (f32 out)
TRACE = [False]
LAST_RESULT = [None]
_STATE = {}


def _build_nc(mode):
    out_bf16 = mode == "bf16o"
    y_dt = BF16 if out_bf16 else F32

    nc = bacc.Bacc(
        "TRN2", target_bir_lowering=False, debug=False, num_devices=N_CORES
    )

    xt_d = nc.dram_tensor("xt", (P, NI, CHUNK), BF16, kind="ExternalInput")
    w_d = nc.dram_tensor("w", (P, NI, C), BF16, kind="ExternalInput")
    sc_d = nc.dram_tensor("sc", (P, NI + NT), F32, kind="ExternalInput")
    # y is packed partition-major ([p, j, c] = row j*P+p); host unpacks
    y_d = nc.dram_tensor("y", (P, NT, C), y_dt, kind="ExternalOutput")

    xt_ap, w_ap, sc_ap, y_ap = xt_d.ap(), w_d.ap(), sc_d.ap(), y_d.ap()
    ADD = mybir.AluOpType.add
    BYP = mybir.AluOpType.bypass

    with tile.TileContext(nc) as tc:
        with (
            tc.tile_pool(name="io", bufs=1) as io,
            tc.tile_pool(name="ps", bufs=1, space="PSUM") as psp,
        ):
            # ---- inputs; xt blocks spread over three queues so scans
            # start as early as the per-queue packet rate allows ----
            # need-ordered streaming: each queue delivers tensors in the
            # order the pipeline consumes them (xt_i before W_i; big W
            # split per block so round 0 is not gated on all of W)
            sc_sb = io.tile([P, NI + NT], F32, name="sc_sb")
            xt_sb = io.tile([P, NI, CHUNK], BF16, name="xt_sb")
            w_sb = io.tile([P, NI, C], BF16, name="w_sb")
            nc.scalar.dma_start(sc_sb[:], sc_ap[:, :])
            nc.sync.dma_start(xt_sb[:, 0, :], xt_ap[:, 0, :])
            nc.gpsimd.dma_start(w_sb[:, 0, :], w_ap[:, 0, :])
            nc.scalar.dma_start(xt_sb[:, 1, :], xt_ap[:, 1, :])
            nc.sync.dma_start(xt_sb[:, 3, :], xt_ap[:, 3, :])
            nc.gpsimd.dma_start(xt_sb[:, 2, :], xt_ap[:, 2, :])
            nc.scalar.dma_start(w_sb[:, 1, :], w_ap[:, 1, :])
            nc.sync.dma_start(w_sb[:, 3, :], w_ap[:, 3, :])
            nc.gpsimd.dma_start(w_sb[:, 2, :], w_ap[:, 2, :])

            # ---- stage A: A^T[ci, t] = halo_ci + cumsum_t x^T[ci, t] ----
            A_sb = io.tile([P, NI, CHUNK], BF16, name="A_sb")
            for i in (0, 1, 3, 2):   # xt arrival order across the queues
                nc.vector.tensor_tensor_scan(
                    A_sb[:, i, :],
                    xt_sb[:, i, :],
                    xt_sb[:, i, :],
                    sc_sb[:, i : i + 1],
                    ADD,
                    BYP,
                )

            # ---- stage Y: psY_j = sum_i A^T[ci, tj]^T @ W[ci, :] ----
            psy = [
                psp.tile([P, C], F32, name=f"psy{j}", tag=f"psy{j}")
                for j in range(NT)
            ]
            # rounds ordered by W-block arrival; the last round (i=2) has
            # the shortest post-arrival chain (no scan behind it)
            I_ORDER = (0, 1, 3)
            I_LAST = 2

            def mm(j, i):
                nc.tensor.matmul(
                    psy[j][:],
                    A_sb[:, i, j * P : (j + 1) * P],
                    w_sb[:, i, :],
                    start=(i == 0),
                    stop=(i == I_LAST),
                )

            y_sb = io.tile([P, NT, C], y_dt, name="y_sb")

            def evict(j):
                scol = sc_sb[:, NI + j : NI + j + 1]
                if j % 2 == 0:
                    nc.scalar.mul(y_sb[:, j, :], psy[j][:], scol)
                else:
                    nc.vector.tensor_scalar_mul(y_sb[:, j, :], psy[j][:], scol)

            for i in I_ORDER:
                mm(0, i)
                mm(1, i)
                mm(2, i)
                mm(3, i)
            for j in range(NT):
                mm(j, I_LAST)
                evict(j)
                if j == 0:
                    nc.sync.dma_start(y_ap[:, 0, :], y_sb[:, 0, :])
                elif j == 1:
                    nc.scalar.dma_start(y_ap[:, 1, :], y_sb[:, 1, :])
                elif j == 3:
                    nc.gpsimd.dma_start(y_ap[:, 2:4, :], y_sb[:, 2:4, :])

    nc.compile()
    return nc


def _get_nc():
    key = MODE[0]
    if key not in _STATE:
        _STATE[key] = _build_nc(key)
    return _STATE[key]


def _prepare_in_maps(x, w_attn, w_proj):
    x = np.asarray(x, dtype=np.float32)
    w_attn = np.asarray(w_attn, dtype=np.float32)
    w_proj = np.asarray(w_proj, dtype=np.float32)
    w = (w_attn[:, 2 * C : 3 * C] @ w_proj).astype(np.float32)
    wpk = np.ascontiguousarray(
        w.reshape(NI, P, C).transpose(1, 0, 2)
    ).astype(bfloat16)

    in_maps = []
    for core in range(N_CORES):
        b, tc = divmod(core, T // CHUNK)
        goff = tc * CHUNK
        chunk = x[b, goff : goff + CHUNK, :]
        # (P, NI, CHUNK): features on partitions, time on the free axis
        xt = np.ascontiguousarray(
            chunk.T.reshape(NI, P, CHUNK).transpose(1, 0, 2)
        ).astype(bfloat16)
        # halo: column-sum of all earlier rows in this batch element
        p = x[b, :goff, :].sum(axis=0, dtype=np.float32) if goff else np.zeros(
            C, np.float32
        )
        # scv[r, tt] = 1/(global_row+1) for row tt*P + r of this chunk
        scale = (1.0 / (goff + np.arange(1, CHUNK + 1))).astype(np.float32)
        sc = np.concatenate(
            [p.reshape(NI, P).T, scale.reshape(NT, P).T], axis=1
        ).astype(np.float32)
        in_maps.append({"xt": xt, "w": wpk, "sc": sc})
    return in_maps


def kernel(x, w_attn, w_proj):
    nc = _get_nc()
    in_maps = _prepare_in_maps(x, w_attn, w_proj)
    res = bass_utils.run_bass_kernel_spmd(
        nc, in_maps, core_ids=list(range(N_CORES)), trace=TRACE[0]
    )
    LAST_RESULT[0] = res
    y = np.empty((B, T, C), np.float32)
    for core in range(N_CORES):
        b, tc = divmod(core, T // CHUNK)
        yp = np.asarray(res.results[core]["y"], dtype=np.float32)
        y[b, tc * CHUNK : (tc + 1) * CHUNK, :] = yp.transpose(1, 0, 2).reshape(
            CHUNK, C
        )
    return y


# revision 12
# speedup vs baseline: 1.6237x; 1.0089x over previous
"""Trainium2 Bass kernel for nn_CausalSelfAttention_74268574482879.

The reference module's attention scores are overwritten by the causal mask
(q/k are discarded), so softmax weights are uniform over positions <= t:
    y = cummean_T(x) @ W,   W = w_attn[:, 1024:1536] @ w_proj  (host-folded)

Distribution: the 4096 rows of (B*T) are split into 8 chunks of 512 rows,
one per NeuronCore.  The only cross-chunk dependency is the column-sum of
all preceding rows in the same batch element; the host passes that tiny
(512,) halo vector per core while slicing the shards.

Per-core dataflow (~40 instructions), tuned for DMA issue rate (the
per-queue descriptor rate, not HBM bandwidth, limits transfers) and for
dependency depth:
  - x^T arrives pre-transposed/packed bf16 (features on partitions), one
    DMA per 128-feature block spread over three trigger queues; W is a
    single 4 KB/partition DMA on the scalar queue
  - 8 half-length tensor_tensor_scan ops (DVE gets blocks 0-1, GpSimd
    blocks 2-3) compute the running column-sum along time, seeded with
    the halo via `initial`, halves chained via initial=prev[:, -1:]
  - 16 bf16 matmuls accumulate psY_j; emission order follows data
    availability (first-half time tiles first, scan block 3 last) so the
    PE never waits long and output row-pairs finish early
  - eviction fuses the deferred 1/(t+1) row scale (per-partition scalar),
    split across Activation/DVE; y leaves as two packed bf16 DMAs that
    the host unpacks/upcasts
"""

import numpy as np
from ml_dtypes import bfloat16

import concourse.bass as bass
import concourse.bacc as bacc
import concourse.mybir as mybir
import concourse.tile as tile
from concourse import bass_utils

N_CORES = 8
B, T, C = 2, 2048, 512
CHUNK = 512               # rows of flattened (B*T) per core
P = 128
NT = CHUNK // P           # 4 row-tiles per chunk
NI = C // P               # 4 col-tiles of the 512 feature dim
H = CHUNK // 2            # half-scan length
F32 = mybir.dt.float32
BF16 = mybir.dt.bfloat16

MODE = ["bf16o"]          # "bf16o" (bf16 out) | "bf16" (f32 out)
TRACE = [False]
LAST_RESULT = [None]
_STATE = {}


def _build_nc(mode):
    out_bf16 = mode == "bf16o"
    y_dt = BF16 if out_bf16 else F32

    nc = bacc.Bacc(
        "TRN2", target_bir_lowering=False, debug=False, num_devices=N_CORES
    )

    xt_d = nc.dram_tensor("xt", (P, NI, CHUNK), BF16, kind="ExternalInput")
    w_d = nc.dram_tensor("w", (P, NI, C), BF16, kind="ExternalInput")
    sc_d = nc.dram_tensor("sc", (P, NI + NT), F32, kind="ExternalInput")
    # y is packed partition-major ([p, j, c] = row j*P+p); host unpacks
    y_d = nc.dram_tensor("y", (P, NT, C), y_dt, kind="ExternalOutput")

    xt_ap, w_ap, sc_ap, y_ap = xt_d.ap(), w_d.ap(), sc_d.ap(), y_d.ap()
    ADD = mybir.AluOpType.add
    BYP = mybir.AluOpType.bypass

    with tile.TileContext(nc) as tc:
        with (
            tc.tile_pool(name="io", bufs=1) as io,
            tc.tile_pool(name="ps", bufs=1, space="PSUM") as psp,
        ):
            # ---- inputs; xt blocks spread over three queues so scans
            # start as early as the per-queue packet rate allows ----
            # need-ordered streaming: each queue delivers tensors in the
            # order the pipeline consumes them (xt_i before W_i; big W
            # split per block so round 0 is not gated on all of W)
            sc_sb = io.tile([P, NI + NT], F32, name="sc_sb")
            xt_sb = io.tile([P, NI, CHUNK], BF16, name="xt_sb")
            w_sb = io.tile([P, NI, C], BF16, name="w_sb")
            nc.scalar.dma_start(sc_sb[:], sc_ap[:, :])
            nc.sync.dma_start(xt_sb[:, 0, :], xt_ap[:, 0, :])
            nc.gpsimd.dma_start(w_sb[:, 0, :], w_ap[:, 0, :])
            nc.scalar.dma_start(xt_sb[:, 1, :], xt_ap[:, 1, :])
            nc.sync.dma_start(xt_sb[:, 3, :], xt_ap[:, 3, :])
            nc.gpsimd.dma_start(xt_sb[:, 2, :], xt_ap[:, 2, :])
            nc.scalar.dma_start(w_sb[:, 1, :], w_ap[:, 1, :])
            nc.sync.dma_start(w_sb[:, 3, :], w_ap[:, 3, :])
            nc.gpsimd.dma_start(w_sb[:, 2, :], w_ap[:, 2, :])

            # ---- stage A: A^T[ci, t] = halo_ci + cumsum_t x^T[ci, t] ----
            A_sb = io.tile([P, NI, CHUNK], BF16, name="A_sb")
            for i in (0, 1, 3, 2):   # xt arrival order across the queues
                nc.vector.tensor_tensor_scan(
                    A_sb[:, i, :],
                    xt_sb[:, i, :],
                    xt_sb[:, i, :],
                    sc_sb[:, i : i + 1],
                    ADD,
                    BYP,
                )

            # ---- stage Y: psY_j = sum_i A^T[ci, tj]^T @ W[ci, :] ----
            psy = [
                psp.tile([P, C], F32, name=f"psy{j}", tag=f"psy{j}")
                for j in range(NT)
            ]
            # rounds ordered by W-block arrival; the last round (i=2) has
            # the shortest post-arrival chain (no scan behind it)
            I_ORDER = (0, 1, 3)
            I_LAST = 2

            def mm(j, i):
                nc.tensor.matmul(
                    psy[j][:],
                    A_sb[:, i, j * P : (j + 1) * P],
                    w_sb[:, i, :],
                    start=(i == 0),
                    stop=(i == I_LAST),
                )

            y_sb = io.tile([P, NT, C], y_dt, name="y_sb")

            def evict(j):
                scol = sc_sb[:, NI + j : NI + j + 1]
                if j % 2 == 0:
                    nc.scalar.mul(y_sb[:, j, :], psy[j][:], scol)
                else:
                    nc.vector.tensor_scalar_mul(y_sb[:, j, :], psy[j][:], scol)

            for i in I_ORDER:
                mm(0, i)
                mm(1, i)
                mm(2, i)
                mm(3, i)
            for j in range(NT):
                mm(j, I_LAST)
                evict(j)
                if j == 1:
                    nc.sync.dma_start(y_ap[:, 0:2, :], y_sb[:, 0:2, :])
                elif j == 3:
                    nc.gpsimd.dma_start(y_ap[:, 2:4, :], y_sb[:, 2:4, :])

    nc.compile()
    return nc


def _get_nc():
    key = MODE[0]
    if key not in _STATE:
        _STATE[key] = _build_nc(key)
    return _STATE[key]


def _prepare_in_maps(x, w_attn, w_proj):
    x = np.asarray(x, dtype=np.float32)
    w_attn = np.asarray(w_attn, dtype=np.float32)
    w_proj = np.asarray(w_proj, dtype=np.float32)
    w = (w_attn[:, 2 * C : 3 * C] @ w_proj).astype(np.float32)
    wpk = np.ascontiguousarray(
        w.reshape(NI, P, C).transpose(1, 0, 2)
    ).astype(bfloat16)

    in_maps = []
    for core in range(N_CORES):
        b, tc = divmod(core, T // CHUNK)
        goff = tc * CHUNK
        chunk = x[b, goff : goff + CHUNK, :]
        # (P, NI, CHUNK): features on partitions, time on the free axis
        xt = np.ascontiguousarray(
            chunk.T.reshape(NI, P, CHUNK).transpose(1, 0, 2)
        ).astype(bfloat16)
        # halo: column-sum of all earlier rows in this batch element
        p = x[b, :goff, :].sum(axis=0, dtype=np.float32) if goff else np.zeros(
            C, np.float32
        )
        # scv[r, tt] = 1/(global_row+1) for row tt*P + r of this chunk
        scale = (1.0 / (goff + np.arange(1, CHUNK + 1))).astype(np.float32)
        sc = np.concatenate(
            [p.reshape(NI, P).T, scale.reshape(NT, P).T], axis=1
        ).astype(np.float32)
        in_maps.append({"xt": xt, "w": wpk, "sc": sc})
    return in_maps


def kernel(x, w_attn, w_proj):
    nc = _get_nc()
    in_maps = _prepare_in_maps(x, w_attn, w_proj)
    res = bass_utils.run_bass_kernel_spmd(
        nc, in_maps, core_ids=list(range(N_CORES)), trace=TRACE[0]
    )
    LAST_RESULT[0] = res
    y = np.empty((B, T, C), np.float32)
    for core in range(N_CORES):
        b, tc = divmod(core, T // CHUNK)
        yp = np.asarray(res.results[core]["y"], dtype=np.float32)
        y[b, tc * CHUNK : (tc + 1) * CHUNK, :] = yp.transpose(1, 0, 2).reshape(
            CHUNK, C
        )
    return y
